# revision 21
# baseline (speedup 1.0000x reference)
"""GCN encoder (2-layer) on 8 Trainium2 NeuronCores.

Strategy: both GCN aggregations run as dense DoubleRow fp8 matmuls on the
tensor engine (2x the bf16 column rate).  The count matrix C = A + I
(20480x20480, 0.16% dense) is materialized host-side in fp8 (small integer
counts -> exact) from the edge list, node-partitioned column-blocks across
the 8 cores.  Layer 1 is computed aggregate-first:

  h   = relu((A_hat @ x) @ W1 + s (x) b1)      s = A_hat @ 1
  out = A_hat @ (h @ W2) + s (x) b2

so the fp8-quantized operand of AGG1 is x itself (pre-scaled by dinv and
S1 host-side, e4m3), the replicated x@W1 is gone, and the small W1/W2
matmuls run on each core's local 2560-node block only.  z2 = dinv*(h@W2)
is quantized to e4m3 on-chip (scale S2 folded into W2), AllGathered in
fp8, and AGG2 also runs DoubleRow.  The D^-1/2 factors fold into per-row/
per-column scalings at PSUM eviction; biases fold in as rank-1 updates.

DMA discipline: the queues are DESCRIPTOR-rate-bound (~155 ns per
descriptor regardless of 512B..4KB payload), so everything is packed
for maximal per-partition line size: A streams as 1 MB double-blocks
(two qp k-slabs packed, 8 KB contiguous per partition per DMA), xq as
1 MB groups (8 KB lines), and z2 in two gather groups (nt 0-2 / nt
3-4) whose preloads read 1.5/1 KB lines.  The sync queue carries only
dependency-free input streams (A/xq/consts), compute-dependent writes
(z2, out) go on the scalar queue, collectives + gathered-z2 reads go
on the gpsimd queue with the reads deferred one mmw chain so the ring
never head-blocks on a collective sem.  nt=4's A blocks (plus the
last nt=3 pair) stay SBUF-resident from AGG1 and are reused by AGG2;
AGG2 also streams into a second ring in the space freed by xq, and
the n=4 column work for qp-pairs 0..5 is prepended from pins to cover
the phase transition.  A ~200-matmul warmup spin holds the PE p-state
at full clock through the DMA-bound opening.  A 64-byte dummy
collective fires at kernel start to absorb inter-core launch skew.
"""

import sys

sys.path.insert(0, "/opt/trn_rl_repo")

import numpy as np

N_REAL = 20000
NCORES = 8
RBLK = 2500          # real nodes per core
BLK = 2560           # padded nodes per core (20 * 128)
NPAD = NCORES * BLK  # 20480
CIN = 256
CHID = 256
COUT = 128
P = 128
KT = NPAD // 512     # 40 k-tiles over nodes
NP2 = KT // 4        # 10 qp-pair A stream blocks per column tile
NT = BLK // 512      # 5 n-tiles over a core's node block
W4 = 452             # nt=4 tile width: real nodes only (2048+452 = 2500)
S1 = 32.0            # fp8 scale for x (folded out via W1' = W1/S1)
S2 = 32.0            # fp8 scale for z2 (folded in via W2' = W2*S2,
                     # folded out host-side: out /= S2, b2' = b2*S2)

_compiled = None


def _build_nc():
    import concourse.bass as bass  # noqa: F401
    import concourse.mybir as mybir
    import concourse.tile as tile
    from concourse import bacc
    from contextlib import ExitStack

    f16 = mybir.dt.float16
    f8 = mybir.dt.float8e4
    f32 = mybir.dt.float32
    Alu = mybir.AluOpType
    DR = mybir.MatmulPerfMode.DoubleRow

    nc = bacc.Bacc("TRN2", target_bir_lowering=False, debug=False,
                   num_devices=NCORES)

    # External I/O, pre-tiled so every big DMA is one contiguous block
    # with 8 KB per partition line (one descriptor per partition).
    xqT = nc.dram_tensor("xqT", [KT // 8, P, 32, CIN], f8,
                         kind="ExternalInput")
    W1 = nc.dram_tensor("W1", [P, CIN // P, CHID], f16, kind="ExternalInput")
    W2 = nc.dram_tensor("W2", [P, CHID // P, COUT], f16, kind="ExternalInput")
    Ab = nc.dram_tensor("Ab", [NP2, NT - 1, P, 16, 512], f8,
                        kind="ExternalInput")
    Ab4 = nc.dram_tensor("Ab4", [NP2, P, 16, W4], f8,
                         kind="ExternalInput")
    zfill = nc.dram_tensor("zfill", [P, COUT], f8, kind="ExternalInput")
    sbc = nc.dram_tensor("sbc", [P, BLK], f16, kind="ExternalInput")
    dbc = nc.dram_tensor("dbc", [P, BLK], f16, kind="ExternalInput")
    dz2 = nc.dram_tensor("dz2", [P, BLK // P], f32, kind="ExternalInput")
    b1c = nc.dram_tensor("b1c", [P, CHID // P], f32, kind="ExternalInput")
    b2c = nc.dram_tensor("b2c", [P, COUT // P], f32, kind="ExternalInput")
    outT = nc.dram_tensor("outT", [P, 1, BLK], f16, kind="ExternalOutput")

    # Internal DRAM (collective buffers).  Two gather groups (nt 0-2 and
    # nt 3-4): group A fires after sweep 2 and covers everything the
    # prepend needs; group B fires right after AGG1.  Grouping keeps the
    # preload lines chunky (1.5/1 KB vs 512 B for per-nt gathers).
    bar_l = nc.dram_tensor("bar_l", [1, 16], f32)
    bar_g = nc.dram_tensor("bar_g", [NCORES, 1, 16], f32,
                           addr_space="Shared")
    z2bA = nc.dram_tensor("z2bA", [P, 3, 4, COUT], f8)
    z2bB = nc.dram_tensor("z2bB", [P, 2, 4, COUT], f8)
    z2gA = nc.dram_tensor("z2gA", [NCORES, P, 3, 4, COUT], f8,
                          addr_space="Shared")
    z2gB = nc.dram_tensor("z2gB", [NCORES, P, 2, 4, COUT], f8,
                          addr_space="Shared")

    # Aggregation k-step q -> (core g, z2-tile t) interleaved t-major so
    # AGG2 can start on gather group A while group B is in flight.
    # Ab's tile axis is host-permuted to this order for both aggregations.
    def q_to_phys(q):
        t, g = divmod(q, NCORES)
        return g * NT + t  # physical global k-tile index

    with tile.TileContext(nc) as tc:
        with ExitStack() as octx:
            const = octx.enter_context(tc.tile_pool(name="const", bufs=1))
            s_sb = const.tile([P, BLK], f16)
            d_sb = const.tile([P, BLK], f16)
            dz2_sb = const.tile([P, BLK // P], f32)
            b1_sb = const.tile([P, CHID // P], f32)
            b2_sb = const.tile([P, COUT // P], f32)
            w1_sb = const.tile([P, CIN // P, CHID], f16)
            w2_sb = const.tile([P, CHID // P, COUT], f16)
            zf_sb = const.tile([P, COUT], f8)
            warm_sb = const.tile([P, 2, 512], f8)
            warm_pool = octx.enter_context(
                tc.tile_pool(name="warm_ps", bufs=1, space="PSUM"))
            warm_ps = warm_pool.tile([P, 512], f32, name="warmps")

            # consts are first read ~35us in; their DMAs are emitted after
            # the first A block so xq[0]/A[0] lead the queues at launch.
            def load_late_consts():
                nc.sync.dma_start(w1_sb[:], W1[:])
                nc.sync.dma_start(w2_sb[:], W2[:])
                nc.sync.dma_start(d_sb[:], dbc[:])
                nc.sync.dma_start(s_sb[:], sbc[:])
                nc.sync.dma_start(dz2_sb[:], dz2[:])
                nc.sync.dma_start(b1_sb[:], b1c[:])
                nc.sync.dma_start(b2_sb[:], b2c[:])
                nc.sync.dma_start(zf_sb[:], zfill[:])

            # A-tile stream pool for AGG1; bufs=11 double-blocks keeps the
            # 10 nt=4 pairs plus nt=3's last pair valid after AGG1 so AGG2
            # reuses them from SBUF instead of re-streaming 11.5 MB.  The
            # AGG2 stream pool opens HERE (before the xq pool) so its
            # address range is disjoint from xq and its first blocks can
            # stream during AGG1's tail.
            a1_kxn = octx.enter_context(tc.tile_pool(name="a1_kxn",
                                                     bufs=11))
            a2_kxn = octx.enter_context(tc.tile_pool(name="a2_kxn",
                                                     bufs=4))
            z2pre_pool = octx.enter_context(
                tc.tile_pool(name="z2pre", bufs=NCORES))
            # xq pool closes after AGG1 so its 5.2 MB is reusable.
            xq_cm = tc.tile_pool(name="xqsb", bufs=KT // 8)
            xq_pool = xq_cm.__enter__()
            xq_tiles = {}   # mt//8 -> group tile [P, 32, CIN]
            zg = {}         # (t, g) -> [P, 4, COUT] z2 AP slice
            pins = {}       # (qp2, nt) -> A double-tile handle (AGG2 reuse)

            # ---- Phases 1-3 fused: AGG1 + MMW1 + MMW2 + gathers ----------
            with ExitStack() as ctx:
                a1_ps = ctx.enter_context(
                    tc.tile_pool(name="a1_ps", bufs=2, space="PSUM"))
                agx_pool = ctx.enter_context(tc.tile_pool(name="agx",
                                                          bufs=2))
                h_pool = ctx.enter_context(tc.tile_pool(name="hsb",
                                                        bufs=2))
                mmh_ps = ctx.enter_context(
                    tc.tile_pool(name="mmh_ps", bufs=1, space="PSUM"))
                mmz_ps = ctx.enter_context(
                    tc.tile_pool(name="mmz_ps", bufs=1, space="PSUM"))
                a1_red = ctx.enter_context(tc.tile_pool(name="a1_red",
                                                        bufs=2))
                z2q_pool = ctx.enter_context(tc.tile_pool(name="z2q",
                                                          bufs=2))

                # PE p-state warmup: ~55 dummy 512-col matmuls on a zeroed
                # tile keep the PE continuously busy (and ramped to full
                # clock) through the ~11us DMA-bound opening, so the first
                # real sweeps run at 2.4 GHz instead of 0.65/1.2 GHz.
                nc.gpsimd.memset(warm_sb[:], 0)
                for _ in range(55):
                    nc.tensor.matmul(warm_ps[:], warm_sb[:, :, :P],
                                     warm_sb[:], start=True, stop=True,
                                     perf_mode=DR)

                def load_xq(g8):
                    if g8 not in xq_tiles:
                        xg = xq_pool.tile([P, 32, CIN], f8, tag="xq")
                        if not xq_tiles:
                            # first group quarter-split for a fast PE start
                            for sub in range(4):
                                nc.sync.dma_start(
                                    xg[:, 8 * sub:8 * sub + 8, :],
                                    xqT[g8, :, 8 * sub:8 * sub + 8, :])
                        else:
                            nc.sync.dma_start(xg[:], xqT[g8])
                        xq_tiles[g8] = xg

                # nt=0 iterates pairs stride-4 so each 1 MB xq group is
                # first touched just before it is needed, spreading the
                # 5.2 MB xq load across the whole first pass.  start/stop
                # flags are positional (first/last emitted matmul).
                p2_nt0 = [p for r in range(4) for p in range(r, NP2, 4)]
                pending_mmw = [None]

                def emit_pre(grp):
                    # preload the gathered z2 tiles.  The parked DGEs
                    # head-block the gpsimd sequencer on the collective
                    # sem, but the ring is otherwise idle between the two
                    # gathers so nothing is delayed.
                    gt, nts = ((z2gA, (0, 1, 2)) if grp == 0
                               else (z2gB, (3, 4)))
                    for g in range(NCORES):
                        zp = z2pre_pool.tile([P, len(nts), 4, COUT],
                                             f8, tag=f"z2pre{grp}",
                                             bufs=NCORES)
                        nc.gpsimd.dma_start(zp[:], gt[g])
                        for i, t in enumerate(nts):
                            zg[(t, g)] = zp[:, i]

                for nt in range(NT):
                    n0 = nt * 512
                    W = W4 if nt == NT - 1 else 512
                    psums = [a1_ps.tile([P, W], f32, name=f"a1ps{m}")
                             for m in range(2)]
                    order = p2_nt0 if nt == 0 else list(range(NP2))
                    for qi2, qp2 in enumerate(order):
                        at = a1_kxn.tile([P, 16, W], f8, tag="a1A")
                        if nt == NT - 1 or (nt == NT - 2
                                            and qp2 == NP2 - 1):
                            pins[(qp2, nt)] = at
                        if nt == 0:
                            # earliest-deadline interleave of the xq group
                            # loads with the A stream so the PE ramps
                            # without waiting on either
                            for q in range(4 * qp2, 4 * qp2 + 4):
                                load_xq(q_to_phys(q) // 8)
                        if nt == 0 and qi2 == 0:
                            # quarter-split the first A block so the first
                            # matmuls start as soon as 256 KB lands
                            for sub in range(4):
                                nc.sync.dma_start(
                                    at[:, 4 * sub:4 * sub + 4, :],
                                    Ab[qp2, nt, :, 4 * sub:4 * sub + 4, :])
                        else:
                            nc.sync.dma_start(
                                at[:],
                                Ab4[qp2] if nt == NT - 1 else Ab[qp2, nt])
                        for sq in range(2):
                            qp = 2 * qp2 + sq
                            for half in range(2):
                                q = 2 * qp + half
                                mt = q_to_phys(q)
                                xg = xq_tiles[mt // 8]
                                mo = (mt % 8) * 4
                                for jp in range(2):
                                    for m in range(2):
                                        nc.tensor.matmul(
                                            psums[m][:],
                                            xg[:, mo + 2 * jp:
                                               mo + 2 * jp + 2,
                                               m * P:(m + 1) * P],
                                            at[:, sq * 8 + half * 4 + 2 * jp:
                                               sq * 8 + half * 4 + 2 * jp + 2,
                                               :],
                                            start=(qi2 == 0 and sq == 0
                                                   and half == 0
                                                   and jp == 0),
                                            stop=(qi2 == NP2 - 1 and sq == 1
                                                  and half == 1 and jp == 1),
                                            perf_mode=DR)
                        if nt == 0 and qi2 == 0:
                            load_late_consts()
                            nc.gpsimd.collective_compute(
                                "AllGather", mybir.AluOpType.bypass,
                                ins=[bar_l[:]], outs=[bar_g[:]],
                                replica_groups=[list(range(NCORES))])
                        if qi2 == 1 and pending_mmw[-1] is not None:
                            pending_mmw[-1]()
                            pending_mmw[-1] = None

                    def mmw_chain(nt=nt, n0=n0, W=W,
                                 psums=psums):
                        # evict: aggxT = d * psum (fp16, ch-major), S1 folded
                        # into W1' host-side
                        agx = agx_pool.tile([P, 2, W], f16, tag="agx")
                        for m in range(2):
                            nc.vector.tensor_mul(agx[:, m, :], psums[m][:],
                                                 d_sb[:, n0:n0 + W])

                        # MMW1: h = relu(aggxT.T @ W1' + b1 (x) s), ch-major
                        ht = h_pool.tile([P, 2, W], f16, tag="h")
                        for mo in range(2):
                            psh = mmh_ps.tile([P, W], f32, name=f"mmh{mo}")
                            for kk in range(2):
                                nc.tensor.matmul(
                                    psh[:],
                                    w1_sb[:, kk, mo * P:(mo + 1) * P],
                                    agx[:, kk, :],
                                    start=(kk == 0), stop=(kk == 1))
                            tmp = a1_red.tile([P, W], f32, tag="a1t")
                            nc.vector.scalar_tensor_tensor(
                                tmp[:], s_sb[:, n0:n0 + W],
                                b1_sb[:, mo:mo + 1], psh[:],
                                op0=Alu.mult, op1=Alu.add)
                            nc.vector.tensor_scalar_max(ht[:, mo, :], tmp[:],
                                                        0.0)

                        # MMW2: z2q = e4m3(d * (h.T @ W2')), node-major fp8.
                        # nt=4's last node slice is 68 wide; the remaining 60
                        # partitions of zq are filled from the zeros const so
                        # no uninitialized fp8 (potential NaN) reaches AGG2.
                        zq = z2q_pool.tile([P, 4, COUT], f8, tag="z2q")
                        ps3 = mmz_ps.tile([P, 4 * COUT], f32, name="mmz")
                        for ns in range(4):
                            nw = min(P, W - ns * P)
                            if nw < P:
                                # 32-aligned zero-fill first; the valid [0:nw)
                                # write below overwrites the overlap
                                nc.vector.tensor_copy(zq[64:P, ns, :],
                                                      zf_sb[64:P, :])
                            psl = ps3[:nw, ns * COUT:(ns + 1) * COUT]
                            for mo in range(2):
                                nc.tensor.matmul(
                                    psl, ht[:, mo, ns * P:ns * P + nw],
                                    w2_sb[:, mo],
                                    start=(mo == 0), stop=(mo == 1))
                            nc.vector.tensor_scalar_mul(
                                zq[:nw, ns, :], psl,
                                dz2_sb[:nw, nt * 4 + ns:nt * 4 + ns + 1])
                        # scalar-queue write: keeps this compute-dependent DMA
                        # from head-blocking the A stream on the sync queue
                        if nt < 3:
                            nc.scalar.dma_start(z2bA[:, nt], zq[:])
                        else:
                            nc.scalar.dma_start(z2bB[:, nt - 3], zq[:])

                        if nt == 2:
                            nc.gpsimd.collective_compute(
                                "AllGather", mybir.AluOpType.bypass,
                                ins=[z2bA[:]], outs=[z2gA[:]],
                                replica_groups=[list(range(NCORES))])
                            emit_pre(0)
                        elif nt == NT - 1:
                            nc.gpsimd.collective_compute(
                                "AllGather", mybir.AluOpType.bypass,
                                ins=[z2bB[:]], outs=[z2gB[:]],
                                replica_groups=[list(range(NCORES))])
                            emit_pre(1)
                    pending_mmw[-1:] = [mmw_chain]
                if pending_mmw[-1] is not None:
                    pending_mmw[-1]()
            xq_cm.__exit__(None, None, None)

            # ---- Phase 4: outT = d*contract(z2q, C) + b2' (x) s ----------
            # Hand-rolled k-outer loop: one PSUM bank per n-tile, so the
            # first gathered z2 tiles start compute while group B is
            # still in flight.  Pinned A blocks come from SBUF.
            with ExitStack() as ctx:
                a2_red = ctx.enter_context(tc.tile_pool(name="a2_red",
                                                        bufs=2))
                a2_ps = ctx.enter_context(
                    tc.tile_pool(name="a2_ps", bufs=1, space="PSUM"))
                # second AGG2 stream ring in the space freed by the xq
                # pool: together with a2_kxn it gives ~4 qp-pairs of
                # prefetch depth.
                a3_kxn = ctx.enter_context(tc.tile_pool(name="a3_kxn",
                                                        bufs=5))

                psums = [a2_ps.tile([P, W4 if n == NT - 1 else 512], f32,
                                    name=f"a2ps{n}")
                         for n in range(NT)]

                def zt_for(q):
                    t, g = divmod(q, NCORES)
                    return zg[(t, g)]

                def evict_out(n):
                    n0 = n * 512
                    W = W4 if n == NT - 1 else 512
                    tmp = a2_red.tile([P, W], f32, tag="a2t")
                    osb = a2_red.tile([P, W], f16, tag="a2o")
                    nc.vector.tensor_mul(tmp[:], psums[n][:],
                                         d_sb[:, n0:n0 + W])
                    nc.vector.scalar_tensor_tensor(
                        osb[:], s_sb[:, n0:n0 + W],
                        b2_sb[:, 0:1], tmp[:],
                        op0=Alu.mult, op1=Alu.add)
                    nc.scalar.dma_start(outT[:, 0, n0:n0 + W], osb[:])

                # prepended pinned work: the n=4 column for qp-pairs 0..5
                # (t=0..2 z2 from gather group A) runs first from
                # SBUF-resident pins, covering the phase transition while
                # the AGG2 A stream fills
                NPRE2 = 6
                for qp2 in range(NPRE2):
                    at = pins[(qp2, NT - 1)]
                    for sq in range(2):
                        qp = 2 * qp2 + sq
                        for half in range(2):
                            q = 2 * qp + half
                            zt = zt_for(q)
                            for jp in range(2):
                                nc.tensor.matmul(
                                    psums[NT - 1][:],
                                    zt[:, 2 * jp:2 * jp + 2, :],
                                    at[:, sq * 8 + half * 4 + 2 * jp:
                                       sq * 8 + half * 4 + 2 * jp + 2, :],
                                    start=(qp2 == 0 and sq == 0
                                           and half == 0 and jp == 0),
                                    stop=False, perf_mode=DR)

                for qp2 in range(NP2):
                    pool = a2_kxn if qp2 % 2 == 0 else a3_kxn
                    tg = "a2A" if pool is a2_kxn else "a3A"
                    ats = []
                    for n in range(NT):
                        if (qp2, n) in pins:
                            ats.append(pins[(qp2, n)])
                            continue
                        wn = W4 if n == NT - 1 else 512
                        at = pool.tile([P, 16, wn], f8, tag=tg)
                        nc.sync.dma_start(
                            at[:], Ab4[qp2] if n == NT - 1 else Ab[qp2, n])
                        ats.append(at)
                    for sq in range(2):
                        qp = 2 * qp2 + sq
                        if qp < 2 * NP2 - 1:
                            for half in range(2):
                                q = 2 * qp + half
                                zt = zt_for(q)
                                for jp in range(2):
                                    for n in range(NT):
                                        if n == NT - 1 and qp2 < NPRE2:
                                            continue  # prepended above
                                        nc.tensor.matmul(
                                            psums[n][:],
                                            zt[:, 2 * jp:2 * jp + 2, :],
                                            ats[n][:, sq * 8 + half * 4
                                                   + 2 * jp:
                                                   sq * 8 + half * 4
                                                   + 2 * jp + 2, :],
                                            start=(q == 0 and jp == 0
                                                   and n < NT - 1),
                                            stop=False,
                                            perf_mode=DR)
                        else:
                            # final qp runs n-major and evicts each psum the
                            # moment its accumulation closes, overlapping the
                            # output drain with the remaining matmuls
                            for n in range(NT):
                                for half in range(2):
                                    q = 2 * qp + half
                                    zt = zt_for(q)
                                    for jp in range(2):
                                        nc.tensor.matmul(
                                            psums[n][:],
                                            zt[:, 2 * jp:2 * jp + 2, :],
                                            ats[n][:, sq * 8 + half * 4
                                                   + 2 * jp:
                                                   sq * 8 + half * 4
                                                   + 2 * jp + 2, :],
                                            start=False,
                                            stop=(half == 1 and jp == 1),
                                            perf_mode=DR)
                                evict_out(n)

    nc.compile()
    return nc


def _preprocess(x, edge_index, W1, b1, W2, b2):
    import ml_dtypes

    x = np.asarray(x, dtype=np.float32)
    edge_index = np.asarray(edge_index)
    W1 = np.asarray(W1, dtype=np.float32)
    b1 = np.asarray(b1, dtype=np.float32)
    W2 = np.asarray(W2, dtype=np.float32)
    b2 = np.asarray(b2, dtype=np.float32)

    row = edge_index[0].astype(np.int64)
    col = edge_index[1].astype(np.int64)

    deg = np.bincount(col, minlength=N_REAL).astype(np.float32) + 1.0
    dinv = 1.0 / np.sqrt(deg)

    idx = np.arange(N_REAL, dtype=np.int64)
    pad_id = (idx // RBLK) * BLK + idx % RBLK  # real -> padded node id

    # Dense count matrix, transposed: CT[src, dst] = A[dst, src] + I
    CT = np.zeros((NPAD, NPAD), dtype=np.uint8)
    np.add.at(CT, (pad_id[row], pad_id[col]), 1)
    CT[pad_id, pad_id] += 1
    assert CT.max() <= 16, "count exceeds exact fp8e4m3 integer range"

    # s[c] = sum_r A_hat[c, r]; dinv at padded positions -> 0
    s_real = dinv * (np.bincount(col, weights=dinv[row],
                                 minlength=N_REAL).astype(np.float32) + dinv)
    s_pad = np.zeros(NPAD, dtype=np.float32)
    s_pad[pad_id] = s_real
    dinv_pad = np.zeros(NPAD, dtype=np.float32)
    dinv_pad[pad_id] = dinv

    # xq = e4m3(S1 * dinv * x), grouped 8 k-tiles per DMA block:
    # [mt//8][p][(mt%8)*4 + j][c] = xq[mt*512 + j*128 + p, c]
    x_pad = np.zeros((NPAD, CIN), dtype=np.float32)
    x_pad[pad_id] = x
    xq_full = np.clip(S1 * dinv_pad[:, None] * x_pad, -240.0, 240.0)
    xqT_t = np.ascontiguousarray(
        xq_full.reshape(KT // 8, 8, 4, P, CIN).transpose(0, 3, 1, 2, 4)
        .reshape(KT // 8, P, 32, CIN)
    ).astype(ml_dtypes.float8_e4m3)

    W1_t = np.ascontiguousarray(
        (W1 / S1).astype(np.float16)
        .reshape(CIN // P, P, CHID).transpose(1, 0, 2))
    W2_t = np.ascontiguousarray(
        (W2 * S2).astype(np.float16)
        .reshape(CHID // P, P, COUT).transpose(1, 0, 2))
    b1_t = np.ascontiguousarray(b1.reshape(CHID // P, P).T)
    b2_t = np.ascontiguousarray((b2 * S2).reshape(COUT // P, P).T)

    in_maps = []
    for g in range(NCORES):
        C_g = CT[:, g * BLK:(g + 1) * BLK]
        # [kt][nt][p][s][n] = C_g[kt*512 + s*128 + p, nt*512 + n],
        # then permute the kt axis into the device's q-order
        # (q -> physical kt = (q % NCORES) * NT + q // NCORES) and pack
        # q-pairs, then qp-pairs: [qp2][nt][p][16][512] (8 KB/partition
        # descriptors - the DMA queues are descriptor-rate-bound).
        perm = [(q % NCORES) * NT + q // NCORES for q in range(KT)]
        A_pack = (
            C_g.reshape(KT, 4, P, NT, 512).transpose(0, 3, 2, 1, 4)[perm]
            .reshape(KT // 2, 2, NT, P, 4, 512).transpose(0, 2, 3, 1, 4, 5)
            .reshape(KT // 2, NT, P, 8, 512))
        A_pack2 = (
            A_pack.reshape(NP2, 2, NT, P, 8, 512)
            .transpose(0, 2, 3, 1, 4, 5)
            .reshape(NP2, NT, P, 16, 512))
        A_t = np.ascontiguousarray(
            A_pack2[:, :NT - 1]).astype(ml_dtypes.float8_e4m3)
        A4_t = np.ascontiguousarray(
            A_pack2[:, NT - 1, :, :, :W4]).astype(ml_dtypes.float8_e4m3)
        s_loc = s_pad[g * BLK:(g + 1) * BLK]
        d_loc = dinv_pad[g * BLK:(g + 1) * BLK]
        s_b = np.ascontiguousarray(
            np.broadcast_to(s_loc, (P, BLK))).astype(np.float16)
        d_b = np.ascontiguousarray(
            np.broadcast_to(d_loc, (P, BLK))).astype(np.float16)
        dz2_t = np.ascontiguousarray(d_loc.reshape(BLK // P, P).T)
        in_maps.append(dict(xqT=xqT_t, W1=W1_t, W2=W2_t, Ab=A_t,
                            Ab4=A4_t, sbc=s_b, dbc=d_b, dz2=dz2_t,
                            b1c=b1_t, b2c=b2_t,
                            zfill=np.zeros((P, COUT),
                                           ml_dtypes.float8_e4m3)))
    return in_maps


def _run(inputs, trace=False):
    global _compiled
    if _compiled is None:
        _compiled = _build_nc()
    nc = _compiled
    from concourse.bass_utils import run_bass_kernel_spmd

    in_maps = _preprocess(**inputs)
    res = run_bass_kernel_spmd(nc, in_maps, list(range(NCORES)), trace=trace)
    out = np.empty((N_REAL, COUT), dtype=np.float32)
    for g in range(NCORES):
        out[g * RBLK:(g + 1) * RBLK] = \
            res.results[g]["outT"][:, 0, :RBLK].T.astype(np.float32) / S2
    return out, res


def kernel(**inputs) -> np.ndarray:
    out, _ = _run(inputs, trace=False)
    return out
